# revision 1
# baseline (speedup 1.0000x reference)
"""RBF kernel layer (retrieval_knn): out = exp(-||x - p||^2) for x [131072, 64]
against 512 prototypes, distributed data-parallel over 8 NeuronCores.

Math: exp(-dist2) = exp(2*S) where S[n,m] = cross[n,m] - p_sq[m]/2 - x_sq[n]/2,
computed entirely in two bf16 hi/lo-split GEMMs accumulating in fp32 PSUM:
  mm1: [xh_t; nxsq_h; nxsq_l; 1; 1].T @ [ph; 1; 1; npsq_h; npsq_l]  (K=68)
  mm2: [xh_t; xl_t].T @ [pl; ph]                                    (K=128)
where x = xh + xl, p = ph + pl (bf16 splits; the dropped xl@pl term is
~2^-18), npsq* = bf16 split of -p_sq/2, nxsq* = bf16 split of -x_sq/2.

x arrives as xhl=[xh|xl] [nshard, 128] bf16 row-major; ONE hardware xbar
DMA-transpose per XCHUNK tiles lands [xh_t; xl_t] directly in SBUF (no PE
transpose, no PSUM staging, no DVE transpose copies). The exp has no
per-tile bias, so one ACTIVATE covers OCHUNK tiles' PSUM banks and one DMA
stores OCHUNK tiles. DMA instruction count is minimized because each HWDGE
dma_start costs the issuing engine ~600 ns of descriptor generation.
"""

import numpy as np

# Problem constants (hardcoded per harness contract; kernel.py is self-contained)
N = 131072
D = 64
M = 512
GAMMA = 1.0
NCORES = 8
NSHARD = N // NCORES  # 16384
P = 128
K1 = D + 4  # mm1 contraction: 64 xh rows + 2 xsq rows + 2 ones rows
LHS_SLOTS = 4  # manual rotation slots for A (ones rows initialized once)
XCHUNK = 8  # x tiles per transposed input DMA
OCHUNK = 4  # output tiles per ACTIVATE + output DMA (PSUM 4-bank group)

_cache = {}


def _build_bass(nshard=NSHARD):
    import concourse.mybir as mybir
    import concourse.tile as tile
    from concourse import bacc

    f32 = mybir.dt.float32
    bf16 = mybir.dt.bfloat16
    nt = nshard // P
    assert nt % XCHUNK == 0 and XCHUNK % OCHUNK == 0

    nc = bacc.Bacc(None, target_bir_lowering=False)
    # pre-transposed on host: [p, i*P + j] = [xh|xl] feature p of point i*P+j
    xhl_d = nc.dram_tensor("xhl", [P, nshard], bf16, kind="ExternalInput")
    # rows (-x_sq/2 hi, -x_sq/2 lo, ones, ones) in bf16, [4, i*P+p] layout
    nxsq_d = nc.dram_tensor("nxsq", [4, nt * P], bf16, kind="ExternalInput")
    rhs1_d = nc.dram_tensor("rhs1", [K1, M], bf16, kind="ExternalInput")
    rhs2_d = nc.dram_tensor("rhs2", [2 * D, M], bf16, kind="ExternalInput")
    out_d = nc.dram_tensor("out", [nshard, M], f32, kind="ExternalOutput")

    with tile.TileContext(nc) as tc:
        with (
            tc.tile_pool(name="singles", bufs=1) as singles,
            tc.tile_pool(name="outp", bufs=5) as outp,
            tc.tile_pool(name="ps_o", bufs=2, space="PSUM") as ps_o,
        ):
            rhs1_sb = singles.tile([K1, M], bf16)
            nc.sync.dma_start(rhs1_sb[:], rhs1_d[:])
            rhs2_sb = singles.tile([2 * D, M], bf16)
            nc.sync.dma_start(rhs2_sb[:], rhs2_d[:])
            nxsq_sb = singles.tile([4, nt * P], bf16)
            nc.sync.dma_start(nxsq_sb[:], nxsq_d[:])

            # A slots [68, 128]: rows 0..63 = xh_t, 64..67 =
            # [-x_sq/2 hi; -x_sq/2 lo; 1; 1] (copied per tile from the
            # host-packed nxsq rows; start partition 64 is AP-legal).
            a_slots = []
            for j in range(LHS_SLOTS):
                A_sb = singles.tile([K1, P], bf16, name=f"A{j}")
                a_slots.append(A_sb)

            # x arrives pre-transposed; all 4 MB stays resident in SBUF for
            # the whole kernel. Chunked into XCHUNK-tile copy DMAs (fully
            # contiguous per partition) so compute starts after the first.
            X_all = singles.tile([P, nt * P], bf16)
            for c in range(nt // XCHUNK):
                cs = slice(c * XCHUNK * P, (c + 1) * XCHUNK * P)
                nc.sync.dma_start(X_all[:, cs], xhl_d[:, cs])

            for i in range(nt):
                k = i % OCHUNK
                if k == 0:
                    o_sb = outp.tile([P, OCHUNK, M], f32, tag="o")
                    psum = ps_o.tile([P, OCHUNK, M], f32, tag="psum")

                T = X_all[:, i * P : (i + 1) * P]
                ts = slice(i * P, (i + 1) * P)
                A = a_slots[i % LHS_SLOTS]
                nc.vector.tensor_copy(A[0:D, :], X_all[0:D, ts])
                nc.vector.tensor_copy(A[D:K1, :], nxsq_sb[:, ts])
                nc.tensor.matmul(
                    psum[:, k, :], A[:], rhs1_sb[:], start=True, stop=False
                )
                nc.tensor.matmul(
                    psum[:, k, :], T, rhs2_sb[:], start=False, stop=True
                )

                if k == OCHUNK - 1:
                    # out = exp(2*S) over all OCHUNK PSUM banks at once
                    nc.scalar.activation(
                        o_sb[:],
                        psum[:],
                        mybir.ActivationFunctionType.Exp,
                        bias=0.0,
                        scale=2.0,
                    )
                    i0 = i - (OCHUNK - 1)
                    dest = out_d[i0 * P : (i0 + OCHUNK) * P, :].rearrange(
                        "(t p) m -> p t m", t=OCHUNK
                    )
                    nc.sync.dma_start(dest, o_sb[:])

    nc.finalize()
    return nc


def _get_nc():
    if "nc" not in _cache:
        _cache["nc"] = _build_bass()
    return _cache["nc"]


def _prep_core_arrays(x, prototypes, nshard):
    """Build per-core host arrays (xhl row-major, nxsq, rhs1/rhs2)."""
    import ml_dtypes

    bf = ml_dtypes.bfloat16
    x = np.ascontiguousarray(np.asarray(x, dtype=np.float32))
    prototypes = np.ascontiguousarray(np.asarray(prototypes, dtype=np.float32))

    xh = x.astype(bf)
    xl = (x - xh.astype(np.float32)).astype(bf)
    # [128, N]: rows 0..63 = xh features, 64..127 = xl features
    xhl_t = np.ascontiguousarray(
        np.concatenate([xh, xl], axis=1).T
    )

    nxsq = (-0.5 * (x.astype(np.float64) ** 2).sum(axis=1)).astype(np.float32)
    nxh = nxsq.astype(bf)
    nxl = (nxsq - nxh.astype(np.float32)).astype(bf)

    pt = prototypes.T.astype(np.float32)  # [64, 512]
    ph = pt.astype(bf)
    pl = (pt - ph.astype(np.float32)).astype(bf)

    p_sq = (prototypes.astype(np.float64) ** 2).sum(axis=1)  # [512]
    t = (-0.5 * p_sq).astype(np.float32)
    th = t.astype(bf)
    tl = (t - th.astype(np.float32)).astype(bf)

    ones = np.ones((1, M), dtype=bf)
    # row order matches A: [xh_t rows; nxsq h/l rows; ones rows]
    rhs1 = np.ascontiguousarray(
        np.concatenate([ph, ones, ones, th[None, :], tl[None, :]], axis=0)
    )  # [68, 512] bf16
    rhs2 = np.ascontiguousarray(np.concatenate([pl, ph], axis=0))  # [128, 512]

    ncores = x.shape[0] // nshard
    in_maps = []
    for s in range(ncores):
        sl = slice(s * nshard, (s + 1) * nshard)
        ones_row = np.ones(nshard, dtype=bf)
        nxsq_r = np.ascontiguousarray(
            np.stack([nxh[sl], nxl[sl], ones_row, ones_row], axis=0)
        )
        in_maps.append(
            {
                "xhl": np.ascontiguousarray(xhl_t[:, sl]),
                "nxsq": nxsq_r,
                "rhs1": rhs1,
                "rhs2": rhs2,
            }
        )
    return in_maps


def _prep_inputs(x, prototypes):
    return _prep_core_arrays(x, prototypes, NSHARD)


def _run(inputs, trace=False):
    from concourse.bass_utils import run_bass_kernel_spmd

    in_maps = _prep_inputs(inputs["x"], inputs["prototypes"])
    nc = _get_nc()
    res = run_bass_kernel_spmd(
        nc, in_maps, core_ids=list(range(NCORES)), trace=trace
    )
    out = np.concatenate([r["out"] for r in res.results], axis=0)
    return out, res


def kernel(**inputs) -> np.ndarray:
    out, _ = _run(inputs, trace=False)
    return out



# revision 2
# speedup vs baseline: 1.5006x; 1.5006x over previous
"""RBF kernel layer (retrieval_knn): out = exp(-||x - p||^2) for x [131072, 64]
against 512 prototypes, distributed data-parallel over 8 NeuronCores.

v2 design ([m,n] orientation, single fp16 GEMM, bf16 output):
  out[m, n] = exp(2*S[m, n] - p_sq[m]),  S = cross - x_sq/2
computed as ONE fp16 matmul per 512-column tile with K=66:
  lhsT = [p_t(64 rows); 1; 1]  (stationary, per 128-prototype tile)
  rhs  = [x_t(64 rows); nxsq_h; nxsq_l]  (x features + fp16 hi/lo of -x_sq/2)
The exact -p_sq[m] rides in the ACTIVATE's per-partition bias AP (f32), and
scale=2.0 turns PSUM S into exp(2S - p_sq) in one pass, emitted directly as
bf16 (halves output DMA bytes vs f32; host upconverts + transposes).

Engine budget per core (16384 points, 512 protos): ScalarE exp is the
bottleneck: 65536 elem/lane / 1.2GHz + 32 ACT overheads ~= 63us. DMA moves
2.2MB in + 16.8MB out ~= 50us. PE streams 128 matmuls of 512 cols ~= 27-55us
(HAM-dependent). DVE is idle (prototypes stationary => no lhsT rebuilds).
x_sq/p_sq are computed from the QUANTIZED fp16 inputs, so the kernel is the
exact RBF of (x16, p16): error ~ 2|x-p|*q_rms, small precisely where the
output is large.
"""

import numpy as np

# Problem constants (hardcoded per harness contract; kernel.py is self-contained)
N = 131072
D = 64
M = 512
GAMMA = 1.0
NCORES = 8
NSHARD = N // NCORES  # 16384
P = 128
K1 = D + 2  # contraction: 64 x rows + 2 (-x_sq/2 hi/lo vs ones) rows
MT = M // P  # 4 prototype tiles
NT = NSHARD // 512  # 32 column chunks of 512 points
OCHUNK = 4  # PSUM banks per ACTIVATE + output DMA (double-buffered 4+4)

_cache = {}


def _build_bass(nshard=NSHARD):
    import concourse.mybir as mybir
    import concourse.tile as tile
    from concourse import bacc

    f32 = mybir.dt.float32
    f16 = mybir.dt.float16
    bf16 = mybir.dt.bfloat16

    nc = bacc.Bacc(None, target_bir_lowering=False)
    # x pre-transposed on host: rows 0..63 = features, 64/65 = -x_sq/2 hi/lo
    xr_d = nc.dram_tensor("xr", [K1, nshard], f16, kind="ExternalInput")
    # prototypes transposed: rows 0..63 = features, 64/65 = ones
    lhs_d = nc.dram_tensor("lhs", [K1, M], f16, kind="ExternalInput")
    # npsq[p, t] = -p_sq[t*128 + p] (f32, exact)
    npsq_d = nc.dram_tensor("npsq", [P, MT], f32, kind="ExternalInput")
    # output transposed: out_t[m, n]; host converts to [n, m] f32
    out_d = nc.dram_tensor("out", [M, nshard], bf16, kind="ExternalOutput")

    with tile.TileContext(nc) as tc:
        with (
            tc.tile_pool(name="singles", bufs=1) as singles,
            tc.tile_pool(name="outp", bufs=4) as outp,
            tc.tile_pool(name="ps_o", bufs=2, space="PSUM") as ps_o,
        ):
            lhs_sb = singles.tile([K1, M], f16)
            nc.sync.dma_start(lhs_sb[:], lhs_d[:])
            npsq_sb = singles.tile([P, MT], f32)
            nc.sync.dma_start(npsq_sb[:], npsq_d[:])

            # x stays resident in SBUF (2.2MB); 4 chunked DMAs so compute
            # starts after the first lands.
            X_sb = singles.tile([K1, nshard], f16)
            XCH = 4096
            for ci in range(nshard // XCH):
                cs = slice(ci * XCH, (ci + 1) * XCH)
                nc.sync.dma_start(X_sb[:, cs], xr_d[:, cs])

            for mt in range(MT):
                lhs_ap = lhs_sb[:, mt * P : (mt + 1) * P]
                bias_ap = npsq_sb[:, mt : mt + 1]
                for c in range(NT):
                    k = c % OCHUNK
                    if k == 0:
                        psum = ps_o.tile([P, OCHUNK, 512], f32, tag="psum")
                        o_sb = outp.tile([P, OCHUNK, 512], bf16, tag="o")
                    nc.tensor.matmul(
                        psum[:, k, :],
                        lhs_ap,
                        X_sb[:, c * 512 : (c + 1) * 512],
                        start=True,
                        stop=True,
                    )
                    if k == OCHUNK - 1:
                        # out = exp(2*S - p_sq) over all OCHUNK banks
                        nc.scalar.activation(
                            o_sb[:],
                            psum[:],
                            mybir.ActivationFunctionType.Exp,
                            bias=bias_ap,
                            scale=2.0,
                        )
                        c0 = c - (OCHUNK - 1)
                        dest = out_d[
                            mt * P : (mt + 1) * P, c0 * 512 : (c + 1) * 512
                        ].rearrange("p (t m) -> p t m", t=OCHUNK)
                        nc.sync.dma_start(dest, o_sb[:])

    nc.finalize()
    return nc


def _get_nc():
    if "nc" not in _cache:
        _cache["nc"] = _build_bass()
    return _cache["nc"]


def _prep_core_arrays(x, prototypes, nshard):
    """Per-core host arrays: xr [66, nshard] f16, lhs [66, 512] f16, npsq."""
    x = np.ascontiguousarray(np.asarray(x, dtype=np.float32))
    prototypes = np.ascontiguousarray(np.asarray(prototypes, dtype=np.float32))

    x16 = x.astype(np.float16)
    p16 = prototypes.astype(np.float16)

    # squared norms of the QUANTIZED values (kernel computes exact RBF of
    # the fp16 inputs), split hi/lo in fp16 for the GEMM rows
    nxsq = (-0.5 * (x16.astype(np.float64) ** 2).sum(axis=1)).astype(np.float32)
    nxh = nxsq.astype(np.float16)
    nxl = (nxsq - nxh.astype(np.float32)).astype(np.float16)

    psq = (p16.astype(np.float64) ** 2).sum(axis=1)  # [512]
    npsq = np.ascontiguousarray(
        (-psq.astype(np.float32)).reshape(MT, P).T
    )  # [128, 4]

    ones = np.ones((1, M), dtype=np.float16)
    lhs = np.ascontiguousarray(
        np.concatenate([p16.T, ones, ones], axis=0)
    )  # [66, 512]

    ncores = x.shape[0] // nshard
    in_maps = []
    for s in range(ncores):
        sl = slice(s * nshard, (s + 1) * nshard)
        xr = np.empty((K1, nshard), dtype=np.float16)
        xr[:D] = x16[sl].T
        xr[D] = nxh[sl]
        xr[D + 1] = nxl[sl]
        in_maps.append({"xr": xr, "lhs": lhs, "npsq": npsq})
    return in_maps


def _prep_inputs(x, prototypes):
    return _prep_core_arrays(x, prototypes, NSHARD)


def _run(inputs, trace=False):
    from concourse.bass_utils import run_bass_kernel_spmd

    in_maps = _prep_inputs(inputs["x"], inputs["prototypes"])
    nc = _get_nc()
    res = run_bass_kernel_spmd(
        nc, in_maps, core_ids=list(range(NCORES)), trace=trace
    )
    out = np.empty((N, M), dtype=np.float32)
    for s, r in enumerate(res.results):
        # r["out"] is [512, 16384] bf16 -> [16384, 512] f32
        out[s * NSHARD : (s + 1) * NSHARD] = r["out"].astype(np.float32).T
    return out, res


def kernel(**inputs) -> np.ndarray:
    out, _ = _run(inputs, trace=False)
    return out
